# revision 4
# baseline (speedup 1.0000x reference)
"""Trainium2 Bass kernel for nn_Attention_58574763983246.

Computation (per batch element b, data-parallel over 8 NeuronCores):
    q      = x @ kernel                      [T, D]
    s      = q @ x^T                         [T, T]   (scores, i x j)
    m_j    = sum_{i<=j} s_ij / T             (column mean of triu(s))
    w_ij   = exp(s_ij - m_j) for i <= j else 0
    ctx_j  = sum_i w_ij * x_i / sum_i w_ij   [T, D]

Implementation notes:
  - scores + q computed in fp32 on the PE (exact exp-overflow/NaN threshold
    matching vs the fp32 reference; margins are ~1e-2, f32r noise is too big)
  - column means via cumsum trick: sum_{i<=j} s_ij == cumsum(q)_j . x_j,
    evaluated with a DVE prefix scan + an all-ones matmul reduction
  - triangular masking with additive -inf BEFORE exp (inf*0 would NaN)
  - e = exp(...) stored as float32r; context matmul (e^T @ [x | 1]) runs in
    f32r at full PE rate; the appended ones column yields the denominators
  - normalization folded into the output copy (reciprocal + scalar-mul)
"""

import os

import numpy as np

B, T, D = 8, 4096, 256
NT = T // 128  # 32 row tiles
NJ = T // 512  # 8 column tiles

_cache = {}


def _build():
    import concourse.bacc as bacc
    import concourse.mybir as mybir
    import concourse.tile as tile

    f32 = mybir.dt.float32
    f32r = mybir.dt.float32r
    add = mybir.AluOpType.add
    bypass = mybir.AluOpType.bypass

    nc = bacc.Bacc("TRN2", target_bir_lowering=False, debug=False)
    x_in = nc.dram_tensor("x", [T, D], f32r, kind="ExternalInput")
    k_in = nc.dram_tensor("kernel", [D, D], f32, kind="ExternalInput")
    id_in = nc.dram_tensor("ident", [128, 128], f32, kind="ExternalInput")
    mk_in = nc.dram_tensor("maskneg", [128, 128], f32, kind="ExternalInput")
    out = nc.dram_tensor("out", [T, D], f32, kind="ExternalOutput")
    out_ap = out.ap()

    with tile.TileContext(nc) as tc:
        with (
            tc.tile_pool(name="consts", bufs=1) as consts,
            tc.tile_pool(name="big", bufs=1) as big,
            tc.tile_pool(name="e", bufs=2) as epool,
            tc.tile_pool(name="eraw", bufs=2) as erawpool,
            tc.tile_pool(name="o", bufs=4) as opool,
            tc.tile_pool(name="r", bufs=4) as rpool,
        ):
            ksb = consts.tile([128, 2, D], f32)
            nc.sync.dma_start(out=ksb[:], in_=k_in.ap().rearrange("(h p) d -> p h d", p=128))
            idt = consts.tile([128, 128], f32)
            nc.sync.dma_start(out=idt[:], in_=id_in[:])
            msk = consts.tile([128, 128], f32)
            nc.sync.dma_start(out=msk[:], in_=mk_in[:])
            onesM = consts.tile([128, 128], f32)
            nc.vector.memset(onesM[:], 1.0)

            xa = big.tile([128, NT, D + 2], f32r)  # row-major x, ones col, pad col (f32r needs even free dim)
            nc.sync.dma_start(
                out=xa[:, :, 0:D], in_=x_in.ap().rearrange("(n p) d -> p n d", p=128)
            )
            nc.vector.memset(xa[:, :, D : D + 2].bitcast(f32), 1.0)

            xT = big.tile([128, 2, T], f32)  # x^T, d-major (2 chunks of 128 dims)
            qT = big.tile([128, 2, T], f32)  # (x @ kernel)^T, d-major
            negm = big.tile([128, T], f32)  # -column_mean, replicated over partitions

            # ---------- phase A: transposes, q, column means ----------
            with (
                tc.tile_pool(name="tmp", bufs=1) as tpool,
                tc.tile_pool(name="psA", bufs=2, space="PSUM") as psA,
            ):
                for I in range(NT):
                    for h in range(2):
                        tp = psA.tile([128, 128], f32, tag="tr")
                        nc.tensor.transpose(
                            tp[:], xa[:, I, h * 128 : (h + 1) * 128].bitcast(f32), idt[:]
                        )
                        nc.vector.tensor_copy(xT[:, h, I * 128 : (I + 1) * 128], tp[:])

                for dh in range(2):
                    for ic in range(NJ):
                        qp = psA.tile([128, 512], f32, tag="q")
                        for ch in range(2):
                            nc.tensor.matmul(
                                qp[:],
                                ksb[:, ch, dh * 128 : (dh + 1) * 128],
                                xT[:, ch, ic * 512 : (ic + 1) * 512],
                                start=(ch == 0),
                                stop=(ch == 1),
                            )
                        nc.vector.tensor_copy(qT[:, dh, ic * 512 : (ic + 1) * 512], qp[:])

                tmp0 = tpool.tile([128, T], f32)
                tmp1 = tpool.tile([128, T], f32)
                for h, tmp in ((0, tmp0), (1, tmp1)):
                    nc.vector.tensor_tensor_scan(
                        tmp[:], qT[:, h, :], qT[:, h, :], 0.0, add, bypass
                    )
                    nc.vector.tensor_mul(tmp[:], tmp[:], xT[:, h, :])
                for jc in range(NJ):
                    mp = psA.tile([128, 512], f32, tag="m")
                    for h, tmp in ((0, tmp0), (1, tmp1)):
                        nc.tensor.matmul(
                            mp[:],
                            onesM[:],
                            tmp[:, jc * 512 : (jc + 1) * 512],
                            start=(h == 0),
                            stop=(h == 1),
                        )
                    nc.vector.tensor_scalar_mul(
                        negm[:, jc * 512 : (jc + 1) * 512], mp[:], -1.0 / T
                    )

            # ---------- main loop: scores -> exp -> context ----------
            with (
                tc.tile_pool(name="ps_s", bufs=1, space="PSUM") as sp_pool,
                tc.tile_pool(name="ps_c", bufs=1, space="PSUM") as cp_pool,
            ):
                es_stash = {}
                ctx_tiles = {}
                started = set()

                def emit_scores_exp(J, g):
                    sp = sp_pool.tile([128, 4, 512], f32, tag="s")
                    for t in range(4):
                        I = 4 * g + t
                        for ch in range(2):
                            nc.tensor.matmul(
                                sp[:, t, :],
                                qT[:, ch, I * 128 : (I + 1) * 128],
                                xT[:, ch, J * 512 : (J + 1) * 512],
                                start=(ch == 0),
                                stop=(ch == 1),
                            )
                    for t in range(4):
                        nc.vector.tensor_add(
                            sp[:, t, :], sp[:, t, :], negm[:, J * 512 : (J + 1) * 512]
                        )
                    if g == J:  # diagonal group: mask below-diagonal with -inf
                        for t in range(4):
                            nc.vector.tensor_add(
                                sp[:, t, t * 128 : (t + 1) * 128],
                                sp[:, t, t * 128 : (t + 1) * 128],
                                msk[:],
                            )
                    # exp in full fp32 (exact overflow->inf threshold), then an
                    # exact 2^-64 rescale into f32r: bounds sum(e*x) away from
                    # fp32 overflow; the factor cancels in the normalization.
                    eraw = erawpool.tile([128, 4 * 512], f32, tag="eraw")
                    nc.scalar.activation(
                        eraw[:],
                        sp[:].rearrange("p a b -> p (a b)"),
                        mybir.ActivationFunctionType.Exp,
                    )
                    es = epool.tile([128, 4 * 512], f32r, tag="e")
                    nc.vector.tensor_scalar_mul(es[:], eraw[:], 2.0 ** -64)
                    es_stash[(J, g)] = es

                def emit_ctx(J, g):
                    es = es_stash.pop((J, g))
                    for t in range(4):
                        I = 4 * g + t
                        for c in range(4):
                            jc = 4 * J + c
                            if I > jc:
                                continue
                            key = (J, c)
                            if key not in ctx_tiles:
                                ctx_tiles[key] = cp_pool.tile(
                                    [128, D + 2], f32, tag=f"c{c}", name=f"ctx{c}"
                                )
                            cp = ctx_tiles[key]
                            nc.tensor.matmul(
                                cp[:],
                                es[:, t * 512 + c * 128 : t * 512 + (c + 1) * 128],
                                xa[:, I, :],
                                start=(key not in started),
                                stop=(g == J and t == c),
                            )
                            started.add(key)

                def emit_norm(J):
                    for c in range(4):
                        cp = ctx_tiles.pop((J, c))
                        rec = rpool.tile([128, 1], f32)
                        nc.vector.reciprocal(rec[:], cp[:, D : D + 1])
                        ot = opool.tile([128, D], f32)
                        nc.vector.tensor_scalar_mul(ot[:], cp[:, 0:D], rec[:])
                        r0 = (4 * J + c) * 128
                        nc.sync.dma_start(out=out_ap[r0 : r0 + 128, :], in_=ot[:])

                stages = [(J, g) for J in range(NJ) for g in range(J + 1)]
                prev = None
                for J, g in stages:
                    emit_scores_exp(J, g)
                    if prev is not None:
                        emit_ctx(*prev)
                        if prev[0] != J:
                            emit_norm(prev[0])
                    prev = (J, g)
                emit_ctx(*prev)
                emit_norm(NJ - 1)

    nc.compile()
    return nc


def _get_nc():
    if "nc" not in _cache:
        _cache["nc"] = _build()
    return _cache["nc"]


def kernel(x, kernel):
    os.environ.setdefault("JAX_PLATFORMS", "axon")
    from concourse.bass_utils import run_bass_kernel_spmd

    x = np.asarray(x, dtype=np.float32)
    kernel = np.asarray(kernel, dtype=np.float32)
    assert x.shape == (B, T, D) and kernel.shape == (D, D)

    nc = _get_nc()
    ident = np.eye(128, dtype=np.float32)
    maskneg = np.triu(np.zeros((128, 128), dtype=np.float32)) + np.tril(
        np.full((128, 128), -np.inf, dtype=np.float32), k=-1
    )
    in_maps = [
        {"x": x[b], "kernel": kernel, "ident": ident, "maskneg": maskneg}
        for b in range(B)
    ]
    res = run_bass_kernel_spmd(nc, in_maps, core_ids=list(range(B)))
    return np.stack([res.results[b]["out"] for b in range(B)], axis=0)


# revision 5
# speedup vs baseline: 1.1895x; 1.1895x over previous
"""Trainium2 Bass kernel for nn_Attention_58574763983246.

Computation (per batch element b, data-parallel over 8 NeuronCores):
    q      = x @ kernel                      [T, D]
    s      = q @ x^T                         [T, T]   (scores, i x j)
    m_j    = sum_{i<=j} s_ij / T             (column mean of triu(s))
    w_ij   = exp(s_ij - m_j) for i <= j else 0
    ctx_j  = sum_i w_ij * x_i / sum_i w_ij   [T, D]

Implementation notes:
  - scores + q computed in fp32 on the PE (exact exp-overflow/NaN threshold
    matching vs the fp32 reference; margins are ~1e-2, f32r noise is too big)
  - column means via cumsum trick: sum_{i<=j} s_ij == cumsum(q)_j . x_j,
    evaluated with a DVE prefix scan + an all-ones matmul reduction
  - triangular masking with additive -inf BEFORE exp (inf*0 would NaN)
  - e = exp(...) stored as float32r; context matmul (e^T @ [x | 1]) runs in
    f32r at full PE rate; the appended ones column yields the denominators
  - normalization folded into the output copy (reciprocal + scalar-mul)
"""

import os

import numpy as np

B, T, D = 8, 4096, 256
NT = T // 128  # 32 row tiles
NJ = T // 512  # 8 column tiles

_cache = {}


def _build():
    import concourse.bacc as bacc
    import concourse.mybir as mybir
    import concourse.tile as tile

    f32 = mybir.dt.float32
    f32r = mybir.dt.float32r
    add = mybir.AluOpType.add
    bypass = mybir.AluOpType.bypass

    nc = bacc.Bacc("TRN2", target_bir_lowering=False, debug=False)
    x_in = nc.dram_tensor("x", [T, D], f32r, kind="ExternalInput")
    k_in = nc.dram_tensor("kernel", [D, D], f32, kind="ExternalInput")
    id_in = nc.dram_tensor("ident", [128, 128], f32, kind="ExternalInput")
    mk_in = nc.dram_tensor("maskneg", [128, 128], f32, kind="ExternalInput")
    out = nc.dram_tensor("out", [T, D], f32, kind="ExternalOutput")
    out_ap = out.ap()

    with tile.TileContext(nc) as tc:
        with (
            tc.tile_pool(name="consts", bufs=1) as consts,
            tc.tile_pool(name="big", bufs=1) as big,
            tc.tile_pool(name="e", bufs=2) as epool,
            tc.tile_pool(name="eraw", bufs=2) as erawpool,
            tc.tile_pool(name="o", bufs=4) as opool,
            tc.tile_pool(name="r", bufs=4) as rpool,
        ):
            ksb = consts.tile([128, 2, D], f32)
            nc.sync.dma_start(out=ksb[:], in_=k_in.ap().rearrange("(h p) d -> p h d", p=128))
            idt = consts.tile([128, 128], f32)
            nc.sync.dma_start(out=idt[:], in_=id_in[:])
            msk = consts.tile([128, 128], f32)
            nc.sync.dma_start(out=msk[:], in_=mk_in[:])
            onesM = consts.tile([128, 128], f32)
            nc.vector.memset(onesM[:], 1.0)

            xa = big.tile([128, NT, D + 2], f32r)  # row-major x, ones col, pad col (f32r needs even free dim)
            nc.sync.dma_start(
                out=xa[:, :, 0:D], in_=x_in.ap().rearrange("(n p) d -> p n d", p=128)
            )
            nc.vector.memset(xa[:, :, D : D + 2].bitcast(f32), 1.0)

            xT = big.tile([128, 2, T], f32)  # x^T, d-major (2 chunks of 128 dims)
            qT = big.tile([128, 2, T], f32)  # (x @ kernel)^T, d-major
            negm = big.tile([128, T], f32)  # -column_mean, replicated over partitions

            # ---------- phase A: transposes, q, column means ----------
            with (
                tc.tile_pool(name="tmp", bufs=1) as tpool,
                tc.tile_pool(name="psA", bufs=2, space="PSUM") as psA,
            ):
                for I in range(NT):
                    for h in range(2):
                        tp = psA.tile([128, 128], f32, tag="tr")
                        nc.tensor.transpose(
                            tp[:], xa[:, I, h * 128 : (h + 1) * 128].bitcast(f32), idt[:]
                        )
                        nc.vector.tensor_copy(xT[:, h, I * 128 : (I + 1) * 128], tp[:])

                for dh in range(2):
                    for ic in range(NJ):
                        qp = psA.tile([128, 512], f32, tag="q")
                        for ch in range(2):
                            nc.tensor.matmul(
                                qp[:],
                                ksb[:, ch, dh * 128 : (dh + 1) * 128],
                                xT[:, ch, ic * 512 : (ic + 1) * 512],
                                start=(ch == 0),
                                stop=(ch == 1),
                            )
                        nc.vector.tensor_copy(qT[:, dh, ic * 512 : (ic + 1) * 512], qp[:])

                tmp0 = tpool.tile([128, T], f32)
                tmp1 = tpool.tile([128, T], f32)
                for h, tmp in ((0, tmp0), (1, tmp1)):
                    nc.vector.tensor_tensor_scan(
                        tmp[:], qT[:, h, :], qT[:, h, :], 0.0, add, bypass
                    )
                    nc.vector.tensor_mul(tmp[:], tmp[:], xT[:, h, :])
                for jc in range(NJ):
                    mp = psA.tile([128, 512], f32, tag="m")
                    for h, tmp in ((0, tmp0), (1, tmp1)):
                        nc.tensor.matmul(
                            mp[:],
                            onesM[:],
                            tmp[:, jc * 512 : (jc + 1) * 512],
                            start=(h == 0),
                            stop=(h == 1),
                        )
                    nc.vector.tensor_scalar_mul(
                        negm[:, jc * 512 : (jc + 1) * 512], mp[:], -1.0 / T
                    )

            # ---------- main loop: scores -> exp -> context ----------
            with (
                tc.tile_pool(name="ps_s", bufs=1, space="PSUM") as sp_pool,
                tc.tile_pool(name="ps_c", bufs=1, space="PSUM") as cp_pool,
            ):
                es_stash = {}
                ctx_tiles = {}
                started = set()

                def emit_scores_exp(J, g):
                    sp = sp_pool.tile([128, 4, 512], f32, tag="s")
                    for t in range(4):
                        I = 4 * g + t
                        for ch in range(2):
                            nc.tensor.matmul(
                                sp[:, t, :],
                                qT[:, ch, I * 128 : (I + 1) * 128],
                                xT[:, ch, J * 512 : (J + 1) * 512],
                                start=(ch == 0),
                                stop=(ch == 1),
                            )
                    for t in range(4):
                        nc.vector.tensor_add(
                            sp[:, t, :], sp[:, t, :], negm[:, J * 512 : (J + 1) * 512]
                        )
                    if g == J:  # diagonal group: mask below-diagonal with -inf
                        for t in range(4):
                            nc.vector.tensor_add(
                                sp[:, t, t * 128 : (t + 1) * 128],
                                sp[:, t, t * 128 : (t + 1) * 128],
                                msk[:],
                            )
                    # exp in full fp32 (exact overflow->inf threshold), then an
                    # exact 2^-40 rescale into f32r: bounds sum(e*x) away from
                    # fp32 overflow; the factor cancels in the normalization.
                    eraw = erawpool.tile([128, 4 * 512], f32, tag="eraw")
                    nc.scalar.activation(
                        eraw[:],
                        sp[:].rearrange("p a b -> p (a b)"),
                        mybir.ActivationFunctionType.Exp,
                    )
                    es = epool.tile([128, 4 * 512], f32r, tag="e")
                    nc.vector.tensor_scalar_mul(es[:], eraw[:], 2.0 ** -40)
                    es_stash[(J, g)] = es

                def emit_ctx(J, g):
                    es = es_stash.pop((J, g))
                    for t in range(4):
                        I = 4 * g + t
                        for c in range(4):
                            jc = 4 * J + c
                            if I > jc:
                                continue
                            key = (J, c)
                            if key not in ctx_tiles:
                                ctx_tiles[key] = cp_pool.tile(
                                    [128, D + 2], f32, tag=f"c{c}", name=f"ctx{c}"
                                )
                            cp = ctx_tiles[key]
                            nc.tensor.matmul(
                                cp[:],
                                es[:, t * 512 + c * 128 : t * 512 + (c + 1) * 128],
                                xa[:, I, :],
                                start=(key not in started),
                                stop=(g == J and t == c),
                            )
                            started.add(key)

                def emit_norm(J):
                    for c in range(4):
                        cp = ctx_tiles.pop((J, c))
                        rec = rpool.tile([128, 1], f32)
                        nc.vector.reciprocal(rec[:], cp[:, D : D + 1])
                        ot = opool.tile([128, D], f32)
                        nc.vector.tensor_scalar_mul(ot[:], cp[:, 0:D], rec[:])
                        r0 = (4 * J + c) * 128
                        nc.sync.dma_start(out=out_ap[r0 : r0 + 128, :], in_=ot[:])

                stages = [(J, g) for J in range(NJ) for g in range(J + 1)]
                prev = None
                for J, g in stages:
                    emit_scores_exp(J, g)
                    if prev is not None:
                        emit_ctx(*prev)
                        if prev[0] != J:
                            emit_norm(prev[0])
                    prev = (J, g)
                emit_ctx(*prev)
                emit_norm(NJ - 1)

    nc.compile()
    return nc


def _get_nc():
    if "nc" not in _cache:
        _cache["nc"] = _build()
    return _cache["nc"]


def kernel(x, kernel):
    os.environ.setdefault("JAX_PLATFORMS", "axon")
    from concourse.bass_utils import run_bass_kernel_spmd

    x = np.asarray(x, dtype=np.float32)
    kernel = np.asarray(kernel, dtype=np.float32)
    assert x.shape == (B, T, D) and kernel.shape == (D, D)

    nc = _get_nc()
    ident = np.eye(128, dtype=np.float32)
    maskneg = np.triu(np.zeros((128, 128), dtype=np.float32)) + np.tril(
        np.full((128, 128), -np.inf, dtype=np.float32), k=-1
    )
    in_maps = [
        {"x": x[b], "kernel": kernel, "ident": ident, "maskneg": maskneg}
        for b in range(B)
    ]
    res = run_bass_kernel_spmd(nc, in_maps, core_ids=list(range(B)))
    return np.stack([res.results[b]["out"] for b in range(B)], axis=0)
